# revision 1
# baseline (speedup 1.0000x reference)
"""ALBERT attention (B=2, S=2048, D=1024, H=16, K=64) on 8 TRN2 NeuronCores.

Sharding: core c = (b, g) with b = c // 4 (batch), g = c % 4 (head group of 4
heads). Each core computes output[b, :, 4g:4g+4, :] — outputs are disjoint, so
no collectives are needed; the host slices inputs per core and stitches the
8 per-core outputs back together.

Host-side prep: x is shipped transposed ([D, S], bf16) so both projection
operands are in natural matmul layout, and keys with attention_mask == 0 are
compacted away entirely — they contribute exactly 0 to the context
(exp(-1e4) underflows), so only unmasked to_tensor rows go to the device,
padded to a 128 multiple and re-masked.

Per-core pipeline:
  - projections (bf16 in, f32 PSUM accumulate): QT [2-head 128, S],
    per-head KT [128, S] zero-padded so logits run full-128 contraction
    (K=64-contraction matmuls stream at half rate), and VT -> PE-transposed
    into V [t, (1|V_h) * 4] with a leading ones column per head.
  - per head, per 128-key tile: logits LT [128, F] into PSUM (float32r),
    ScalarE computes ET = exp(0.125 * LT + mask_bias) straight out of PSUM
    (mask bias is per-partition = per key; no max subtraction needed:
    |logits/8| <= ~3 and masked keys underflow to exactly 0, matching the
    reference softmax), then context Cacc[65, F] += [1|V]^T @ ET, so row 0
    of Cacc accumulates the softmax denominators.
  - normalize in C^T layout: GpSimd broadcasts the sums row, VectorE does
    reciprocal_approx_fast + multiply; each head's [64, S] C^T DMAs out and
    the host transposes during unshard. The normalize tail of head h-1 is
    interleaved into head h's key loop so it hides under the exp-bound
    steady state.
"""

import ml_dtypes
import numpy as np

import concourse.bass as bass
import concourse.tile as tile
from concourse import bacc, mybir
from concourse.bass_utils import run_bass_kernel_spmd

F32 = mybir.dt.float32
F32R = mybir.dt.float32r
BF16 = mybir.dt.bfloat16

B, S, D, H, K = 2, 2048, 1024, 16, 64
NH = 4            # heads per core
HK = NH * K       # 256
NCORES = 8
DC = D // 128     # 8 contraction chunks
FT = S // 128     # 16 f tiles
FC = S // 512     # 4 f chunks
NEG = -10000.0


def build_nc(t_tiles: int):
    """Build the per-core Bass graph. t_tiles = number of 128-row key tiles
    (16 when dense; fewer when the host compacts masked-out keys away)."""
    T = t_tiles * 128
    # 512-wide chunks of T (last may be 128/256/384) for the KT/VT projections
    tchunks = [(c, min(512, T - c)) for c in range(0, T, 512)]

    nc = bacc.Bacc("TRN2", target_bir_lowering=False, debug=False,
                   num_devices=NCORES)

    xf_d = nc.dram_tensor("xf", [D, S], BF16, kind="ExternalInput").ap()
    xt_d = nc.dram_tensor("xt", [D, T], BF16, kind="ExternalInput").ap()
    wq_d = nc.dram_tensor("wq", [D, HK], BF16, kind="ExternalInput").ap()
    wk_d = nc.dram_tensor("wk", [D, HK], BF16, kind="ExternalInput").ap()
    wv_d = nc.dram_tensor("wv", [D, HK], BF16, kind="ExternalInput").ap()
    # bias columns: [bq0 bq1 bk0 bk1 bv0 bv1] (hk-tile halves of each bias)
    bias_d = nc.dram_tensor("bias", [128, 6], F32, kind="ExternalInput").ap()
    # additive key mask, tiled: mask_d[p, i] = maskadd[i*128 + p]
    mask_d = nc.dram_tensor("mask", [128, t_tiles], F32,
                            kind="ExternalInput").ap()
    ident_d = nc.dram_tensor("ident", [128, 128], F32,
                             kind="ExternalInput").ap()
    out_d = nc.dram_tensor("out", [NH, K, S], F32,
                       kind="ExternalOutput").ap()

    with tile.TileContext(nc) as tc:
        with (
            tc.sbuf_pool(name="const", bufs=1) as const_pool,
            tc.sbuf_pool(name="persist", bufs=1) as persist_pool,
        ):
            bias_sb = const_pool.tile([128, 6], F32)
            mask_sb = const_pool.tile([128, t_tiles], F32)
            ident_sb = const_pool.tile([128, 128], F32)

            qt_sb = [persist_pool.tile([128, S], BF16, name=f"qt{i}")
                     for i in range(2)]
            # per-head KT, zero-padded to full 128 contraction rows: K_h
            # occupies rows 64*(h%2)..+64 (matching qt's row layout), rest 0
            kt_sb = [persist_pool.tile([128, S], BF16, name=f"kt{i}")
                     for i in range(NH)]
            for h in range(NH):
                zo = 64 * (1 - h % 2)
                nc.vector.memset(kt_sb[h][zo:zo + 64, :], 0.0)
            # V with a ones column in front of each head's 64 columns,
            # one tile per T-tile so consumers can start before all of V
            # is transposed
            v_sb = [persist_pool.tile([128, NH * 65], F32R, name=f"v{i}")
                    for i in range(t_tiles)]
            for i in range(t_tiles):
                nc.vector.memset(
                    v_sb[i].rearrange("p (h c) -> p h c", c=65)[:, :, 0:1]
                    .bitcast(F32), 1.0)


            # ---------------- projections ----------------
            # QT[hk, s] = sum_d wq[d, hk] * xf[d, s]  (and same for KT/VT)
            with (
                tc.sbuf_pool(name="xfp", bufs=8) as xfp_pool,
                tc.sbuf_pool(name="xtp", bufs=8) as xtp_pool,
                tc.sbuf_pool(name="wgt", bufs=1) as wgt_pool,
                tc.sbuf_pool(name="vt", bufs=1) as vt_pool,
            ):
              with tc.psum_pool(name="pp", bufs=8) as pp_pool:
                  # DMA issue order matters: each dma_start costs ~1us of
                  # sequencer issue time, so the tiles gating the first
                  # matmuls (xf chunk d + wq chunk d) go out first,
                  # interleaved; K/V weights and constants follow.
                  wq_t, wk_t, wv_t = [], [], []
                  xf_t, xt_t = [], []
                  for d in range(DC):
                      xf_tile = xfp_pool.tile([128, S], BF16, tag="xf",
                                              name=f"xf{d}")
                      nc.sync.dma_start(xf_tile[:],
                                        xf_d[128 * d:128 * (d + 1), :])
                      xf_t.append(xf_tile)
                      wt = wgt_pool.tile([128, HK], BF16, name=f"wq{d}")
                      nc.sync.dma_start(wt[:], wq_d[128 * d:128 * (d + 1), :])
                      wq_t.append(wt)
                      xt_tile = xtp_pool.tile([128, T], BF16, tag="xt",
                                              name=f"xt{d}")
                      nc.sync.dma_start(xt_tile[:],
                                        xt_d[128 * d:128 * (d + 1), :])
                      xt_t.append(xt_tile)
                      wt = wgt_pool.tile([128, HK], BF16, name=f"wk{d}")
                      nc.sync.dma_start(wt[:], wk_d[128 * d:128 * (d + 1), :])
                      wk_t.append(wt)
                  for d in range(DC):
                      wt = wgt_pool.tile([128, HK], BF16, name=f"wv{d}")
                      nc.sync.dma_start(wt[:], wv_d[128 * d:128 * (d + 1), :])
                      wv_t.append(wt)
                  nc.sync.dma_start(bias_sb[:], bias_d[:])
                  nc.sync.dma_start(mask_sb[:], mask_d[:])
                  nc.sync.dma_start(ident_sb[:], ident_d[:])
                  q_ps = [[pp_pool.tile([128, 512], F32, tag="pp",
                                        name=f"qp{hk}_{s}")
                           for s in range(FC)] for hk in range(2)]
                  for d in range(DC):
                      for hk in range(2):
                          lhs = wq_t[d][:, 128 * hk:128 * (hk + 1)]
                          for s in range(FC):
                              nc.tensor.matmul(
                                  q_ps[hk][s][:],
                                  lhs,
                                  xf_t[d][:, 512 * s:512 * (s + 1)],
                                  start=(d == 0), stop=(d == DC - 1))
                  for hk in range(2):
                      for s in range(FC):
                          nc.vector.tensor_scalar_add(
                              qt_sb[hk][:, 512 * s:512 * (s + 1)],
                              q_ps[hk][s][:], bias_sb[:, hk:hk + 1])

                  # K and V^T projections from xt
                  k_ps = [[pp_pool.tile([128, w], F32, tag="pp",
                                        name=f"kp{hk}_{s}")
                           for s, (c0, w) in enumerate(tchunks)]
                          for hk in range(2)]
                  for d in range(DC):
                      for hk in range(2):
                          lhs = wk_t[d][:, 128 * hk:128 * (hk + 1)]
                          for s, (c0, w) in enumerate(tchunks):
                              nc.tensor.matmul(
                                  k_ps[hk][s][:],
                                  lhs,
                                  xt_t[d][:, c0:c0 + w],
                                  start=(d == 0), stop=(d == DC - 1))
                  for hk in range(2):
                      for s, (c0, w) in enumerate(tchunks):
                          for hh in range(2):
                              nc.vector.tensor_scalar_add(
                                  kt_sb[2 * hk + hh][64 * hh:64 * (hh + 1),
                                                     c0:c0 + w],
                                  k_ps[hk][s][64 * hh:64 * (hh + 1), :],
                                  bias_sb[64 * hh:64 * (hh + 1),
                                          2 + hk:3 + hk])

                  vt_sb = [vt_pool.tile([128, T], F32R, name=f"vt{i}")
                           for i in range(2)]
                  v_ps = [[pp_pool.tile([128, w], F32, tag="pp",
                                        name=f"vp{hk}_{s}")
                           for s, (c0, w) in enumerate(tchunks)]
                          for hk in range(2)]
                  for d in range(DC):
                      for hk in range(2):
                          lhs = wv_t[d][:, 128 * hk:128 * (hk + 1)]
                          for s, (c0, w) in enumerate(tchunks):
                              nc.tensor.matmul(
                                  v_ps[hk][s][:],
                                  lhs,
                                  xt_t[d][:, c0:c0 + w],
                                  start=(d == 0), stop=(d == DC - 1))
                  for hk in range(2):
                      for s, (c0, w) in enumerate(tchunks):
                          nc.vector.tensor_scalar_add(
                              vt_sb[hk][:, c0:c0 + w],
                              v_ps[hk][s][:], bias_sb[:, 4 + hk:5 + hk])

                  # transpose VT [hk, t] -> V [t, hk] via PE (reusing the
                  # projection psum slots), interleaving the ones columns
                  # already memset in v_sb
                  for hk in range(2):
                      for i in range(t_tiles):
                          tr = pp_pool.tile([128, 128], F32, tag="pp",
                                            name=f"vtr{hk}_{i}")
                          nc.tensor.transpose(
                              tr[:],
                              vt_sb[hk][:, 128 * i:128 * (i + 1)].bitcast(F32),
                              ident_sb[:])
                          for hh in range(2):
                              h = 2 * hk + hh
                              nc.vector.tensor_copy(
                                  v_sb[i][:, 65 * h + 1:65 * h + 65],
                                  tr[:, 64 * hh:64 * hh + 64])

            # ---------------- attention, head by head ----------------
            with (
                tc.psum_pool(name="lt", bufs=2) as lt_pool,
                tc.psum_pool(name="cacc", bufs=1) as cacc_pool,
                tc.sbuf_pool(name="et", bufs=8) as et_pool,
                tc.sbuf_pool(name="ct", bufs=2) as ct_pool,
                tc.sbuf_pool(name="rr", bufs=2) as rr_pool,
            ):
                def tail_ops(hprev, ct, nchunk=2):
                    """Normalize head hprev's context in C^T layout:
                    out[hprev] = C^T * (1/sums). The sums row is replicated
                    across the 64 k-partitions on the otherwise-idle GpSimd
                    engine; the final [k, f] -> [f, k] transpose happens on
                    the host during unshard."""
                    rb0 = rr_pool.tile([K + 1, S], F32R, tag="rb0",
                                       name=f"rb0{hprev}")
                    rb = rr_pool.tile([K + 1, S], F32, tag="rb",
                                      name=f"rb{hprev}")
                    ctn = ct_pool.tile([K + 1, S], F32, tag="ctn",
                                       name=f"ctn{hprev}")
                    ops = []
                    for c in range(nchunk):
                        sl = slice(S // nchunk * c, S // nchunk * (c + 1))
                        def bcast(sl=sl):
                            nc.gpsimd.partition_broadcast(
                                rb0[:, sl], ct[0:1, sl])
                        def recip(sl=sl):
                            nc.vector.reciprocal_approx_fast(
                                rb[:, sl], rb0[:, sl].bitcast(F32))
                        def mult_out(sl=sl):
                            # row 0 is sums/sums; the DMA ships rows 1..64
                            # (compute must stay 32-aligned, DMA need not)
                            nc.vector.tensor_mul(
                                ctn[:, sl], ct[:, sl].bitcast(F32),
                                rb[:, sl])
                            nc.sync.dma_start(out_d[hprev][:, sl],
                                              ctn[1:65, sl])
                        ops += [bcast, recip, mult_out]
                    return ops

                pending = []  # remaining tail ops of the previous head
                for h in range(NH):
                    hk = h // 2
                    cacc = cacc_pool.tile([65, S], F32, tag="cacc",
                                          name=f"cacc{h}")
                    for i in range(t_tiles):
                        # kt_sb[h] carries K_h at rows 64*(h%2)..+64 and zeros
                        # elsewhere, so a full-128 contraction against the
                        # 2-head shared qt tile picks out exactly head h.
                        k_lhs = kt_sb[h][:, 128 * i:128 * (i + 1)]
                        ets = []
                        for half in range(2):
                            lt = lt_pool.tile([128, 1024], F32, tag="lt",
                                              name=f"lt{h}_{i}_{half}")
                            for c in range(2):
                                fc = 2 * half + c
                                nc.tensor.matmul(
                                    lt[:, 512 * c:512 * (c + 1)],
                                    k_lhs,
                                    qt_sb[hk][:, 512 * fc:512 * (fc + 1)],
                                    start=True, stop=True)
                            et = et_pool.tile([128, 1024], F32R, tag="et",
                                              name=f"et{h}_{i}_{half}")
                            nc.scalar.activation(
                                et[:], lt[:],
                                mybir.ActivationFunctionType.Exp,
                                bias=mask_sb[:, i:i + 1], scale=0.125)
                            ets.append(et)
                        for fc in range(FC):
                            nc.tensor.matmul(
                                cacc[:, 512 * fc:512 * (fc + 1)],
                                v_sb[i][:, 65 * h:65 * h + 65],
                                ets[fc // 2][:, 512 * (fc % 2):
                                             512 * (fc % 2 + 1)],
                                start=(i == 0), stop=(i == t_tiles - 1),
                                skip_group_check=True)
                        nops = -(-len(pending) // max(1, t_tiles - 1 - i)) \
                            if pending else 0
                        for _ in range(min(nops, len(pending))):
                            pending.pop(0)()

                    while pending:
                        pending.pop(0)()
                    ct = ct_pool.tile([65, S], F32R, tag="ct", name=f"ct{h}")
                    nch = 4 if h == NH - 1 else 2
                    for c in range(nch):
                        sl = slice(S // nch * c, S // nch * (c + 1))
                        nc.vector.tensor_copy(ct[:, sl].bitcast(F32),
                                              cacc[:, sl])
                    pending = tail_ops(h, ct, nch)

                for op in pending:
                    op()

    nc.compile()
    return nc


_NC_CACHE = {}


def _get_nc(t_tiles: int):
    if t_tiles not in _NC_CACHE:
        _NC_CACHE[t_tiles] = build_nc(t_tiles)
    return _NC_CACHE[t_tiles]


def kernel(from_tensor, to_tensor, attention_mask, Wq, bq, Wk, bk, Wv, bv):
    from_tensor = np.asarray(from_tensor, dtype=np.float32)
    to_tensor = np.asarray(to_tensor, dtype=np.float32)
    attention_mask = np.asarray(attention_mask)
    Wq = np.asarray(Wq, dtype=np.float32)
    Wk = np.asarray(Wk, dtype=np.float32)
    Wv = np.asarray(Wv, dtype=np.float32)
    bq = np.asarray(bq, dtype=np.float32)
    bk = np.asarray(bk, dtype=np.float32)
    bv = np.asarray(bv, dtype=np.float32)

    # compact away masked-out keys: they contribute exactly 0 to the
    # context (exp(-1e4) underflows), so only unmasked to_tensor rows are
    # shipped; the tail is padded to a 128 multiple and re-masked.
    mask_np = attention_mask.astype(np.int32)
    idxs = [np.nonzero(mask_np[b])[0] for b in range(B)]
    t_eff = max(1, max(len(ix) for ix in idxs))
    T_pad = min(S, ((t_eff + 127) // 128) * 128)
    t_tiles = T_pad // 128
    nc = _get_nc(t_tiles)

    ident = np.eye(128, dtype=np.float32)
    xt_c = np.zeros((B, D, T_pad), dtype=np.float32)
    maskadd = np.full((B, T_pad), NEG, dtype=np.float32)
    for b in range(B):
        ix = idxs[b]
        xt_c[b, :, :len(ix)] = to_tensor[b].T[:, ix]
        maskadd[b, :len(ix)] = 0.0

    in_maps = []
    for c in range(NCORES):
        b, g = c // 4, c % 4
        hs = slice(NH * g, NH * (g + 1))
        wq = np.ascontiguousarray(Wq[:, hs, :].reshape(D, HK))
        wk = np.ascontiguousarray(Wk[:, hs, :].reshape(D, HK))
        wv = np.ascontiguousarray(Wv[:, hs, :].reshape(D, HK))
        bias = np.stack([
            bq[hs].reshape(HK)[:128], bq[hs].reshape(HK)[128:],
            bk[hs].reshape(HK)[:128], bk[hs].reshape(HK)[128:],
            bv[hs].reshape(HK)[:128], bv[hs].reshape(HK)[128:],
        ], axis=1)
        in_maps.append({
            "xf": np.ascontiguousarray(from_tensor[b].T
                                       .astype(ml_dtypes.bfloat16)),
            "xt": np.ascontiguousarray(xt_c[b].astype(ml_dtypes.bfloat16)),
            "wq": wq.astype(ml_dtypes.bfloat16),
            "wk": wk.astype(ml_dtypes.bfloat16),
            "wv": wv.astype(ml_dtypes.bfloat16),
            "bias": np.ascontiguousarray(bias),
            "mask": np.ascontiguousarray(
                maskadd[b].reshape(t_tiles, 128).T),
            "ident": ident,
        })

    global _LAST_IN_MAPS, _LAST_T_TILES
    _LAST_IN_MAPS = in_maps
    _LAST_T_TILES = t_tiles
    try:
        res = run_bass_kernel_spmd(nc, in_maps, core_ids=list(range(NCORES)))
    except Exception:
        # the axon terminal occasionally reports the device unrecoverable;
        # a reset + retry clears it
        try:
            import ctypes

            lib = ctypes.CDLL("/opt/axon/libaxon_pjrt.so")
            lib.axon_reset.restype = ctypes.c_int64
            lib.axon_reset()
        except Exception:
            pass
        res = run_bass_kernel_spmd(nc, in_maps, core_ids=list(range(NCORES)))

    out = np.empty((B, S, H, K), dtype=np.float32)
    for c in range(NCORES):
        b, g = c // 4, c % 4
        # device ships C^T [head, k, f]; transpose to [f, head, k]
        out[b, :, NH * g:NH * (g + 1), :] = \
            res.results[c]["out"].transpose(2, 0, 1)
    return out



# revision 5
# speedup vs baseline: 1.3107x; 1.3107x over previous
"""ALBERT attention (B=2, S=2048, D=1024, H=16, K=64) on 8 TRN2 NeuronCores.

Sharding: core c = (b, g) with b = c // 4 (batch), g = c % 4 (head group of 4
heads). Each core computes output[b, :, 4g:4g+4, :] — outputs are disjoint, so
no collectives are needed.

Host-side prep: x is shipped transposed ([D, S], bf16); keys with
attention_mask == 0 are compacted away (they contribute exactly 0), padded to
a 128 multiple. Because of the compaction, only the LAST key tile contains
masked (padding) keys, so only its exp() needs the additive-mask bias.

Per-core pipeline (ScalarE exp is the roofline: ~64 ACTs x ~1.33us):
  - projections, weight-stationary, bf16: QT [2-head 128, S] and per-pair
    KT [128, T] (no zero padding; logits contract 64 rows via tile_position),
    V computed DIRECTLY in [t, hk] layout (xt chunks stationary, wv moving)
    so no PE transpose pass is needed. PSUM->SBUF drains ride on DVE
    tensor_scalar_add (+bias).
  - attention, head-sequential, f-half split (PSUM: lt 2x[128,1024] double
    buffered + cacc [65,1024] = 6 banks, 2 spare for overlapped projections):
    per (head, fhalf, ttile): logits LT [128 keys, 1024 f] (K=64
    contraction), ScalarE ET = exp(0.125*LT [+ mask on last tile]) in bf16,
    context Cacc[65, 1024] += [1|V]^T @ ET (row 0 = softmax denominators).
  - output ships UNNORMALIZED: out[h] = [65, S] f32 (row 0 = denom,
    rows 1..65 = C^T). Host divides, transposes, and adds bv (exact since
    probs sum to 1).
"""

import ml_dtypes
import numpy as np

import concourse.bass as bass
import concourse.tile as tile
from concourse import bacc, mybir
from concourse.bass_utils import run_bass_kernel_spmd

F32 = mybir.dt.float32
BF16 = mybir.dt.bfloat16

B, S, D, H, K = 2, 2048, 1024, 16, 64
NH = 4            # heads per core
HK = NH * K       # 256
NCORES = 8
DC = D // 128     # 8 contraction chunks
NEG = -10000.0
FH = 1024         # f-half width


def build_nc(t_tiles: int):
    """Per-core Bass graph. t_tiles = number of 128-row key tiles after
    host-side compaction of masked-out keys."""
    T = t_tiles * 128
    tchunks = [(c, min(512, T - c)) for c in range(0, T, 512)]

    nc = bacc.Bacc("TRN2", target_bir_lowering=False, debug=False,
                   num_devices=NCORES)

    xf_d = nc.dram_tensor("xf", [D, S], BF16, kind="ExternalInput").ap()
    xt_d = nc.dram_tensor("xt", [D, T], BF16, kind="ExternalInput").ap()
    wq_d = nc.dram_tensor("wq", [D, HK], BF16, kind="ExternalInput").ap()
    wk_d = nc.dram_tensor("wk", [D, HK], BF16, kind="ExternalInput").ap()
    wv_d = nc.dram_tensor("wv", [D, HK], BF16, kind="ExternalInput").ap()
    # bias columns: [bq0 bq1 bk0 bk1] (hk-tile halves of bq / bk)
    bias_d = nc.dram_tensor("bias", [128, 4], F32, kind="ExternalInput").ap()
    # additive key mask for the LAST key tile only (all other tiles are
    # fully unmasked after compaction)
    mask_d = nc.dram_tensor("mask", [128, 1], F32, kind="ExternalInput").ap()
    # unnormalized: per head, row 0 = softmax denominators, rows 1..64 = C^T
    out_d = nc.dram_tensor("out", [NH, K + 1, S], F32,
                           kind="ExternalOutput").ap()

    with tile.TileContext(nc) as tc:
        with (
            tc.sbuf_pool(name="const", bufs=1) as const_pool,
            tc.sbuf_pool(name="persist", bufs=1) as persist_pool,
            tc.psum_pool(name="proj", bufs=2) as proj_pool,
            tc.psum_pool(name="lt", bufs=2) as lt_pool,
            tc.psum_pool(name="cacc", bufs=1) as cacc_pool,
            tc.sbuf_pool(name="et", bufs=3) as et_pool,
            tc.sbuf_pool(name="ct", bufs=2) as ct_pool,
        ):
            bias_sb = const_pool.tile([128, 4], F32)
            mask_sb = const_pool.tile([128, 1], F32)

            # big input tiles; views expose [p, chunk, col]
            xf_sb = persist_pool.tile([128, DC * S], BF16, name="xf")
            xt_sb = persist_pool.tile([128, DC * T], BF16, name="xt")
            wq_sb = persist_pool.tile([128, DC * HK], BF16, name="wq")
            wk_sb = persist_pool.tile([128, DC * HK], BF16, name="wk")
            wv_sb = persist_pool.tile([128, DC * HK], BF16, name="wv")
            xf_v = xf_sb.rearrange("p (c s) -> p c s", s=S)
            xt_v = xt_sb.rearrange("p (c s) -> p c s", s=T)
            wq_v = wq_sb.rearrange("p (c s) -> p c s", s=HK)
            wk_v = wk_sb.rearrange("p (c s) -> p c s", s=HK)
            wv_v = wv_sb.rearrange("p (c s) -> p c s", s=HK)

            qt_sb = [persist_pool.tile([128, S], BF16, name=f"qt{i}")
                     for i in range(2)]
            kt_sb = [persist_pool.tile([128, T], BF16, name=f"kt{i}")
                     for i in range(2)]
            # V with a leading ones column per head: [1|V_h0|1|V_h1|...]
            v_sb = [persist_pool.tile([128, NH * 65], BF16, name=f"v{i}")
                    for i in range(t_tiles)]
            for i in range(t_tiles):
                nc.vector.memset(
                    v_sb[i].rearrange("p (h c) -> p h c", c=65)[:, :, 0:1],
                    1.0)

            # ---------------- input DMAs ----------------
            # big fan-out DMAs; issue split across engines so issue cost
            # does not serialize. Order = criticality.
            nc.scalar.dma_start(
                wq_sb.rearrange("p (c s) -> p c s", s=HK),
                wq_d.rearrange("(c p) s -> p c s", p=128))
            nc.scalar.dma_start(
                wk_sb.rearrange("p (c s) -> p c s", s=HK),
                wk_d.rearrange("(c p) s -> p c s", p=128))
            # xt in two d-halves so K/V projections start at the half mark
            xt_src = xt_d.rearrange("(c p) s -> p c s", p=128)
            nc.sync.dma_start(xt_v[:, 0:DC // 2, :], xt_src[:, 0:DC // 2, :])
            nc.sync.dma_start(xt_v[:, DC // 2:, :], xt_src[:, DC // 2:, :])
            # xf f-half 0, in two d-halves
            xf_src = xf_d.rearrange("(c p) s -> p c s", p=128)
            nc.gpsimd.dma_start(xf_v[:, 0:DC // 2, 0:FH],
                                xf_src[:, 0:DC // 2, 0:FH])
            nc.gpsimd.dma_start(xf_v[:, DC // 2:, 0:FH],
                                xf_src[:, DC // 2:, 0:FH])
            nc.scalar.dma_start(
                wv_sb.rearrange("p (c s) -> p c s", s=HK),
                wv_d.rearrange("(c p) s -> p c s", p=128))
            nc.scalar.dma_start(bias_sb[:], bias_d[:])
            nc.scalar.dma_start(mask_sb[:], mask_d[:])
            # xf f-half 1 (only needed once attention on fh0 is running)
            nc.gpsimd.dma_start(xf_v[:, :, FH:S], xf_src[:, :, FH:S])

            def q_proj(hk, fh):
                """QT[hk][:, fh*FH:+FH] <- sum_d wq_d.T @ xf_d, + bq."""
                ps = [proj_pool.tile([128, 512], F32, tag="pp",
                                     name=f"qp{hk}_{fh}_{s}")
                      for s in range(2)]
                for d in range(DC):
                    lhs = wq_v[:, d, 128 * hk:128 * (hk + 1)]
                    for s in range(2):
                        c0 = fh * FH + 512 * s
                        nc.tensor.matmul(ps[s][:], lhs,
                                         xf_v[:, d, c0:c0 + 512],
                                         start=(d == 0), stop=(d == DC - 1))
                for s in range(2):
                    c0 = fh * FH + 512 * s
                    nc.vector.tensor_scalar_add(
                        qt_sb[hk][:, c0:c0 + 512], ps[s][:],
                        bias_sb[:, hk:hk + 1])

            def k_proj(hk):
                """KT[hk] <- sum_d wk_d.T @ xt_d, + bk (no zero padding).
                At most 2 PSUM tiles live at a time (proj_pool bufs=2):
                first two t-chunks run d-outer, the rest d-inner."""
                outer = tchunks[:2]
                ps = [proj_pool.tile([128, w], F32, tag="pp",
                                     name=f"kp{hk}_{s}")
                      for s, (c0, w) in enumerate(outer)]
                for d in range(DC):
                    lhs = wk_v[:, d, 128 * hk:128 * (hk + 1)]
                    for s, (c0, w) in enumerate(outer):
                        nc.tensor.matmul(ps[s][:], lhs,
                                         xt_v[:, d, c0:c0 + w],
                                         start=(d == 0), stop=(d == DC - 1))
                for s, (c0, w) in enumerate(outer):
                    nc.vector.tensor_scalar_add(
                        kt_sb[hk][:, c0:c0 + w], ps[s][:],
                        bias_sb[:, 2 + hk:3 + hk])
                for s, (c0, w) in enumerate(tchunks[2:], start=2):
                    pst = proj_pool.tile([128, w], F32, tag="pp",
                                         name=f"kp{hk}_{s}")
                    for d in range(DC):
                        nc.tensor.matmul(
                            pst[:], wk_v[:, d, 128 * hk:128 * (hk + 1)],
                            xt_v[:, d, c0:c0 + w],
                            start=(d == 0), stop=(d == DC - 1))
                    nc.vector.tensor_scalar_add(
                        kt_sb[hk][:, c0:c0 + w], pst[:],
                        bias_sb[:, 2 + hk:3 + hk])

            def v_proj(t):
                """v_sb[t][:, h*65+1 : h*65+65] <- (xt tile t).T @ wv.
                bv is added on the host (exact: probs sum to 1)."""
                ps = proj_pool.tile([128, HK], F32, tag="pp", name=f"vp{t}")
                for d in range(DC):
                    nc.tensor.matmul(ps[:],
                                     xt_v[:, d, 128 * t:128 * (t + 1)],
                                     wv_v[:, d, :],
                                     start=(d == 0), stop=(d == DC - 1))
                nc.vector.tensor_copy(
                    v_sb[t].rearrange("p (h c) -> p h c", c=65)[:, :, 1:65],
                    ps.rearrange("p (h c) -> p h c", c=64)[:, :, :])

            # critical-path projections first: everything head pair 0 /
            # f-half 0 needs to start the exp pipeline.
            q_proj(0, 0)
            k_proj(0)
            for t in range(t_tiles):
                v_proj(t)

            # ---------------- attention ----------------
            def attention(h, fh):
                hk, zo = h // 2, 64 * (h % 2)
                cacc = cacc_pool.tile([K + 1, FH], F32, tag="cacc",
                                      name=f"cacc{h}_{fh}")
                for t in range(t_tiles):
                    lt = lt_pool.tile([128, FH], F32, tag="lt",
                                      name=f"lt{h}_{fh}_{t}")
                    for s in range(2):
                        c0 = fh * FH + 512 * s
                        nc.tensor.matmul(
                            lt[:, 512 * s:512 * (s + 1)],
                            kt_sb[hk][zo:zo + 64, 128 * t:128 * (t + 1)],
                            qt_sb[hk][zo:zo + 64, c0:c0 + 512],
                            start=True, stop=True)
                    et = et_pool.tile([128, FH], BF16, tag="et",
                                      name=f"et{h}_{fh}_{t}")
                    nc.scalar.activation(
                        et[:], lt[:], mybir.ActivationFunctionType.Exp,
                        bias=(mask_sb[:, 0:1] if t == t_tiles - 1 else 0.0),
                        scale=0.125)
                    for s in range(2):
                        nc.tensor.matmul(
                            cacc[:, 512 * s:512 * (s + 1)],
                            v_sb[t][:, 65 * h:65 * (h + 1)],
                            et[:, 512 * s:512 * (s + 1)],
                            start=(t == 0), stop=(t == t_tiles - 1),
                            skip_group_check=True)
                ct = ct_pool.tile([K + 1, FH], F32, tag="ct",
                                  name=f"ct{h}_{fh}")
                nc.vector.tensor_copy(ct[:], cacc[:])
                nc.gpsimd.dma_start(out_d[h][:, fh * FH:(fh + 1) * FH],
                                    ct[:])

            # interleave: attention (h0, fh0) is runnable immediately; the
            # remaining projections are issued next so the Tile scheduler
            # slots their matmuls into PE gaps while ACT (exp) is the
            # bottleneck engine.
            attention(0, 0)
            k_proj(1)
            q_proj(1, 0)
            attention(1, 0)
            q_proj(0, 1)
            q_proj(1, 1)
            attention(2, 0)
            attention(3, 0)
            attention(0, 1)
            attention(1, 1)
            attention(2, 1)
            attention(3, 1)

    nc.compile()
    return nc


_NC_CACHE = {}


def _get_nc(t_tiles: int):
    if t_tiles not in _NC_CACHE:
        _NC_CACHE[t_tiles] = build_nc(t_tiles)
    return _NC_CACHE[t_tiles]


def kernel(from_tensor, to_tensor, attention_mask, Wq, bq, Wk, bk, Wv, bv):
    from_tensor = np.asarray(from_tensor, dtype=np.float32)
    to_tensor = np.asarray(to_tensor, dtype=np.float32)
    attention_mask = np.asarray(attention_mask)
    Wq = np.asarray(Wq, dtype=np.float32)
    Wk = np.asarray(Wk, dtype=np.float32)
    Wv = np.asarray(Wv, dtype=np.float32)
    bq = np.asarray(bq, dtype=np.float32)
    bk = np.asarray(bk, dtype=np.float32)
    bv = np.asarray(bv, dtype=np.float32)

    # compact away masked-out keys (they contribute exactly 0 to the
    # context); pad to a 128 multiple and re-mask the padding tail.
    mask_np = attention_mask.astype(np.int32)
    idxs = [np.nonzero(mask_np[b])[0] for b in range(B)]
    t_eff = max(1, max(len(ix) for ix in idxs))
    T_pad = min(S, ((t_eff + 127) // 128) * 128)
    t_tiles = T_pad // 128
    nc = _get_nc(t_tiles)

    xt_c = np.zeros((B, D, T_pad), dtype=np.float32)
    maskadd = np.full((B, T_pad), NEG, dtype=np.float32)
    for b in range(B):
        ix = idxs[b]
        xt_c[b, :, :len(ix)] = to_tensor[b].T[:, ix]
        maskadd[b, :len(ix)] = 0.0

    in_maps = []
    for c in range(NCORES):
        b, g = c // 4, c % 4
        hs = slice(NH * g, NH * (g + 1))
        wq = np.ascontiguousarray(Wq[:, hs, :].reshape(D, HK))
        wk = np.ascontiguousarray(Wk[:, hs, :].reshape(D, HK))
        wv = np.ascontiguousarray(Wv[:, hs, :].reshape(D, HK))
        bias = np.stack([
            bq[hs].reshape(HK)[:128], bq[hs].reshape(HK)[128:],
            bk[hs].reshape(HK)[:128], bk[hs].reshape(HK)[128:],
        ], axis=1)
        in_maps.append({
            "xf": np.ascontiguousarray(from_tensor[b].T
                                       .astype(ml_dtypes.bfloat16)),
            "xt": np.ascontiguousarray(xt_c[b].astype(ml_dtypes.bfloat16)),
            "wq": wq.astype(ml_dtypes.bfloat16),
            "wk": wk.astype(ml_dtypes.bfloat16),
            "wv": wv.astype(ml_dtypes.bfloat16),
            "bias": np.ascontiguousarray(bias),
            "mask": np.ascontiguousarray(
                maskadd[b][(t_tiles - 1) * 128:].reshape(128, 1)),
        })

    global _LAST_IN_MAPS, _LAST_T_TILES
    _LAST_IN_MAPS = in_maps
    _LAST_T_TILES = t_tiles
    try:
        res = run_bass_kernel_spmd(nc, in_maps, core_ids=list(range(NCORES)))
    except Exception:
        # the axon terminal occasionally reports the device unrecoverable;
        # a reset + retry clears it
        try:
            import ctypes

            lib = ctypes.CDLL("/opt/axon/libaxon_pjrt.so")
            lib.axon_reset.restype = ctypes.c_int64
            lib.axon_reset()
        except Exception:
            pass
        res = run_bass_kernel_spmd(nc, in_maps, core_ids=list(range(NCORES)))

    out = np.empty((B, S, H, K), dtype=np.float32)
    for c in range(NCORES):
        b, g = c // 4, c % 4
        o = res.results[c]["out"]          # [NH, 65, S]
        ctx = o[:, 1:, :] / o[:, 0:1, :]   # normalize by denominators
        # [NH, K, S] -> [S, NH, K], plus bv
        out[b, :, NH * g:NH * (g + 1), :] = \
            ctx.transpose(2, 0, 1) + bv[NH * g:NH * (g + 1)][None]
    return out
